# revision 74
# baseline (speedup 1.0000x reference)
"""Trainium2 Bass kernel for nn_Attention_53257594471037.

Multi-head attention layer (B=8, N=1024, embed 512 + class 512):
  qk = x[:, :, -512:] @ Wqk + bqk ; q, k = split(qk)      (8 heads, dh=64)
  v  = x @ Wv + bv                                        (8 heads, dv=128)
  out = softmax(q k^T / sqrt(64)) v                       per head
  y  = concat(out) @ Wo + bo
Sharding: data-parallel over batch -- each of the 8 NeuronCores handles one
batch element end to end.  No collectives.

Key device-time optimizations over a straight bf16 implementation:
  - All projection operands are pre-transposed / pre-packed / fp8-hi-lo-split
    on the HOST, so the device spends no PE cycles transposing x and every
    DMA is a maximal-contiguous-line copy in final SBUF layout.
  - The qk, v and y projections run on the PE in fp8e4m3 DoubleRow mode: one
    matmul instruction contracts TWO 128-row k-tiles at half the
    cycles-per-row of bf16 (4x MAC rate).  Accuracy is preserved with a
    3-term error split: x ~ x_hi + x_lo, W ~ W_hi + W_lo (all scaled fp8),
    x@W ~ x_hi@W_hi + x_hi@W_lo + x_lo@W_hi, accumulated in one fp32 PSUM
    group.  Measured on HW this is ~2x MORE accurate than bf16 inputs
    (hi+lo carries ~8.5 effective mantissa bits).
  - Scores: pair 0 stays bf16 (its scores start ~4.3us; no room for extra
    latency).  Pairs 1-3 run scores in fp8 DoubleRow at HALF the bf16
    streaming cost with FULL 4-term hi/lo accuracy: q is stored as a
    [q_hi;q_lo] partition stack (rhs, broadcast across the two DR k-tiles
    via a stride-0 tile axis), k as partition-duplicated [k_hi;k_hi] /
    [k_lo;k_lo] tiles, repacked from the pair-layout quant buffers by
    SBUF->SBUF gpsimd-queue DMAs.  The bqk bias is dropped by the quant
    (hi+lo is exactly unbiased); q-side bias terms are constant over j and
    cancel in softmax, and the k-side term d_j = bq.k_j is restored exactly
    as a per-partition exp bias computed on device by tiny fp8 matmuls +
    PE transposes.  Net: better accuracy than bf16 scores (4.2e-3 vs
    5.7e-3 end-to-end).  PV stays bf16: exp in single fp8 measures 2.6e-2
    end-to-end (fails the 2e-2 gate), and an exp hi/lo split would need a
    second elementwise pass over 8.4M elements (>55us on any engine).
  - Softmax denominators come free from the PV matmul via a 1/SO ones
    column in the augmented V, which also pre-scales the attention output
    for its fp8 hi/lo split.  bv is folded into bo on the host
    (softmax rows sum to 1, so y = out@Wo + (bv@Wo + bo)).
  - Wqk columns are host-permuted so the two blocks gating attention pair 0
    ride the first small DMA; the six projection steps they feed are split
    into term phases pipelined 6-deep across otherwise-idle PSUM banks,
    tracking DMA chunk arrivals.
  - The emission order interleaves scores/exp (the serial ACT spine),
    projections, PV, transposes and the y projection so the PE stream stays
    dependency-fed end to end; elementwise copy-outs are balanced across
    DVE/ACT/Pool per phase; the tail fuses PV(pair 3) + transposes + y
    projection one i-tile apart, ending in small chunks so the closing
    matmul->copy->DMA chain is short.
"""

import os

os.environ.setdefault("MYCRO_LOCAL_CACHE", "1")

import numpy as np
import ml_dtypes

# --- problem constants (hardcoded; kernel.py must be self-contained) ---
B = 8
N = 1024          # tokens
D = 1024          # embed + class feature width
CLS = 512         # class width; qk projection reads x[:, :, -CLS:]
HEADS = 8
DH = 64           # per-head q/k dim
DV = 128          # per-head v dim
SCALE = DH ** -0.5
NT = N // 128     # 8 token tiles
DC = D // 128     # 8 feature chunks
VSTRIDE = 130     # per-head stride in v_aug: 128 data + 1 ones + 1 pad

# fp8e4m3 (IEEE: max finite 240) scales.  Inputs are ~N(0,1); weights are
# ~N(0, fan_in^-1/2).  Chosen so |scaled| stays well under 240.
SX = 2.0 ** 5     # x:   max |x| ~ 5.2  -> ~166
SQK = 2.0 ** 9    # Wqk: max ~ 0.24     -> ~121
SV = 2.0 ** 9     # Wv:  max ~ 0.17     -> ~88
SO = 2.0 ** 5     # attention out (+bv): |.| < ~6 -> ~192
SY = 2.0 ** 9     # Wo:  max ~ 0.17     -> ~88
UNSC_QK = 1.0 / (SX * SQK)
UNSC_V = 1.0 / (SX * SV)
UNSC_Y = 1.0 / (SO * SY)
SQ8_H = 2.0 ** 4   # fp8 q/k quant scale (pairs 1-3 score path)
SBQ_H = 2.0 ** 12  # fp8 bq scale for the d_j = bq.k exp-bias matmul

E4M3 = ml_dtypes.float8_e4m3
BF16 = ml_dtypes.bfloat16

# wqk column m-block order in DRAM/SBUF (m -> sbuf block M2SB[m])
M_ORDER = [0, 4, 1, 5, 2, 6, 3, 7]
M2SB = {m: i for i, m in enumerate(M_ORDER)}

_COMPILED = None  # cached compiled module so repeated kernel() calls reuse it


def _build():
    import concourse.mybir as mybir
    import concourse.tile as tile
    from concourse import bacc

    f32 = mybir.dt.float32
    bf16 = mybir.dt.bfloat16
    fp8 = mybir.dt.float8e4
    DR = mybir.MatmulPerfMode.DoubleRow
    Exp = mybir.ActivationFunctionType.Exp
    Ident = mybir.ActivationFunctionType.Identity
    mult = mybir.AluOpType.mult
    add = mybir.AluOpType.add
    subtract = mybir.AluOpType.subtract

    nc = bacc.Bacc(None, target_bir_lowering=False)

    # fp8 hi/lo pairs, host-split, host-transposed and host-packed into the
    # exact SBUF layout [128, chunk*cols] so every DMA moves maximal
    # contiguous lines.  wqk is additionally split into a 256-col "head"
    # (the permuted m=0,4 blocks that gate attention pair 0) and the rest.
    xc_hi_d = nc.declare_dram_parameter("xTc_hi", [128, 4 * N], fp8, isOutput=False)
    xc_lo_d = nc.declare_dram_parameter("xTc_lo", [128, 4 * N], fp8, isOutput=False)
    xe_hi_d = nc.declare_dram_parameter("xTe_hi", [128, 4 * N], fp8, isOutput=False)
    xe_lo_d = nc.declare_dram_parameter("xTe_lo", [128, 4 * N], fp8, isOutput=False)
    wqkh_hi_d = nc.declare_dram_parameter("wqkh_hi", [128, 4 * 256], fp8, isOutput=False)
    wqkh_lo_d = nc.declare_dram_parameter("wqkh_lo", [128, 4 * 256], fp8, isOutput=False)
    wqkr_hi_d = nc.declare_dram_parameter("wqkr_hi", [128, 4 * 768], fp8, isOutput=False)
    wqkr_lo_d = nc.declare_dram_parameter("wqkr_lo", [128, 4 * 768], fp8, isOutput=False)
    wv_hi_d = nc.declare_dram_parameter("wv_hi", [128, 8 * D], fp8, isOutput=False)
    wv_lo_d = nc.declare_dram_parameter("wv_lo", [128, 8 * D], fp8, isOutput=False)
    wo_hi_d = nc.declare_dram_parameter("wo_hi", [128, 8 * D], fp8, isOutput=False)
    wo_lo_d = nc.declare_dram_parameter("wo_lo", [128, 8 * D], fp8, isOutput=False)
    bqk_d = nc.declare_dram_parameter("bqk_t", [128, 8], f32, isOutput=False)
    bqp8_d = nc.declare_dram_parameter("bqp8", [128, 192], fp8, isOutput=False)
    bo_d = nc.declare_dram_parameter("bo_t", [128, D], bf16, isOutput=False)
    y_d = nc.declare_dram_parameter("y", [N, D], f32, isOutput=True)

    ident_const = nc.inline_tensor(
        np.eye(128, dtype=np.float32).astype(BF16), name="identc"
    )

    with tile.TileContext(nc) as tc:
        with (
            tc.tile_pool(name="persist", bufs=1) as pp,
            tc.tile_pool(name="expsp", bufs=4) as ep,
            tc.tile_pool(name="small", bufs=2) as sp,
            tc.tile_pool(name="yout", bufs=4) as yp,
            tc.tile_pool(name="ps_mm", bufs=2, space="PSUM") as ps_mm,
            tc.tile_pool(name="ps_s", bufs=2, space="PSUM") as ps_s,
            tc.tile_pool(name="ps_o", bufs=2, space="PSUM") as ps_o,
        ):
            # ---------- loads.  Big tensors ride HWDGE (sync) in consumption
            # order; small/late tensors ride the gpsimd SWDGE queue so their
            # prep never blocks the serialized 625ns-per-transfer HWDGE.

            # qk-projection data first (gates everything): the wqk head
            # (m=0,4) and class-x stream in the prologue phases' term order
            wqkh_hi = pp.tile([128, 4, 256], fp8, name="wqkhh")
            wqkh_lo = pp.tile([128, 4, 256], fp8, name="wqkhl")
            wqkr_hi = pp.tile([128, 4, 768], fp8, name="wqkrh")
            wqkr_lo = pp.tile([128, 4, 768], fp8, name="wqkrl")
            xc_hi = pp.tile([128, 4, N], fp8, name="xch")
            xc_lo = pp.tile([128, 4, N], fp8, name="xcl")
            xc_hi_r = xc_hi_d.rearrange("p (c n) -> p c n", c=4)
            xc_lo_r = xc_lo_d.rearrange("p (c n) -> p c n", c=4)
            nc.sync.dma_start(
                out=wqkh_hi[:, :, :], in_=wqkh_hi_d.rearrange("p (c n) -> p c n", c=4)
            )
            # first x chunk rides the gpsimd queue: its SWDGE prep runs in
            # parallel with the HWDGE preps, landing both first-matmul
            # operands sooner
            nc.gpsimd.dma_start(out=xc_hi[:, 0:2, :], in_=xc_hi_r[:, 0:2, :])
            bqk_col = pp.tile([128, 8], f32)
            nc.gpsimd.dma_start(out=bqk_col[:, :], in_=bqk_d[:, :])
            nc.sync.dma_start(out=xc_hi[:, 2:4, :], in_=xc_hi_r[:, 2:4, :])
            nc.sync.dma_start(
                out=wqkh_lo[:, :, :], in_=wqkh_lo_d.rearrange("p (c n) -> p c n", c=4)
            )
            nc.sync.dma_start(out=xc_lo[:, 0:2, :], in_=xc_lo_r[:, 0:2, :])
            nc.sync.dma_start(out=xc_lo[:, 2:4, :], in_=xc_lo_r[:, 2:4, :])
            nc.sync.dma_start(
                out=wqkr_hi[:, :, :], in_=wqkr_hi_d.rearrange("p (c n) -> p c n", c=4)
            )
            nc.sync.dma_start(
                out=wqkr_lo[:, :, :], in_=wqkr_lo_d.rearrange("p (c n) -> p c n", c=4)
            )
            # small late-consumed tensors: after the qk stream so they never
            # delay it, still well before their first use
            ident = pp.tile([128, 128], bf16)
            nc.sync.dma_start(out=ident[:, :], in_=ident_const[:, :])
            # v-projection data (consumed from pair 1 onward), ordered so the
            # hi-hi terms of the first vproj steps unlock earliest
            xe_hi = pp.tile([128, 4, N], fp8, name="xeh")
            nc.sync.dma_start(
                out=xe_hi[:, :, :], in_=xe_hi_d.rearrange("p (c n) -> p c n", c=4)
            )
            wv_hi = pp.tile([128, DC, 1024], fp8, name="wvh")
            nc.sync.dma_start(
                out=wv_hi[:, :, :], in_=wv_hi_d.rearrange("p (c n) -> p c n", c=8)
            )
            wv_lo = pp.tile([128, DC, 1024], fp8, name="wvl")
            nc.sync.dma_start(
                out=wv_lo[:, :, :], in_=wv_lo_d.rearrange("p (c n) -> p c n", c=8)
            )
            xe_lo = pp.tile([128, 4, N], fp8, name="xel")
            nc.sync.dma_start(
                out=xe_lo[:, :, :], in_=xe_lo_d.rearrange("p (c n) -> p c n", c=4)
            )
            # bo rides after the v stream: first consumer is yproj (~90us)
            bo_bc = pp.tile([128, D], bf16)
            nc.sync.dma_start(out=bo_bc[:, :], in_=bo_d[:, :])
            # y-projection data (consumed last)
            wo_hi = pp.tile([128, DC, 1024], fp8, name="woh")
            nc.sync.dma_start(
                out=wo_hi[:, :, :], in_=wo_hi_d.rearrange("p (c n) -> p c n", c=8)
            )
            wo_lo = pp.tile([128, DC, 1024], fp8, name="wol")
            nc.sync.dma_start(
                out=wo_lo[:, :, :], in_=wo_lo_d.rearrange("p (c n) -> p c n", c=8)
            )

            # force the ACT function-table load (1283ns) at t~0.4 via a
            # tiny dummy exp, instead of lazily at ~5.2us where it finishes
            # only ~300ns before the first copy-out needs the ACT engine
            warm1 = pp.tile([128, 1], f32, name="warm1")
            nc.gpsimd.memset(warm1[:, :], 0.0)
            nc.scalar.activation(warm1[:, :], warm1[:, :], Exp, scale=1.0)

            # ---------- qkT[f, n] = (Wqk^T @ x_clsT)/SxSqk + bqk ----------
            # fp8 DoubleRow, 3-term compensated.  Term order within a step is
            # chosen so the earliest steps depend on the earliest DMAs.
            # Pair 0 keeps the original bf16 q/k store (its scores start
            # ~4.3us -- no room for quantize+repack latency).  Pairs 1-3
            # store q/k as scaled fp8 hi/lo instead: their scores then run
            # in DoubleRow at HALF the bf16 streaming cost, with FULL 4-term
            # hi/lo accuracy:
            #   S = [k_hi;k_hi]^T[q_hi;q_lo] + [k_lo;k_lo]^T[q_hi;q_lo]
            # (one DR matmul; k tiles partition-duplicated, q a [hi;lo]
            # partition stack broadcast across the two DR k-tiles).
            # The bqk bias is DROPPED by the quant ops (hi+lo reproduces the
            # unbiased value exactly); q-side bias terms are constant over j
            # and cancel in softmax, and the k-side term d_j = bq.k_j is
            # restored exactly as a per-partition exp bias, computed on
            # device from the pair-layout k8 by tiny DR matmuls.
            qkT = pp.tile([128, 2, N], bf16)
            outT_hi = pp.tile([128, DC, N], fp8, name="outTh")
            outT_lo = pp.tile([128, DC, N], fp8, name="outTl")
            SQ8 = SQ8_H       # q,k fp8 scale: max |q,k| ~5.2 -> ~83
            SBQ = SBQ_H       # bq fp8 scale: max |bq| ~.045 -> ~184
            SC8 = SCALE / (SQ8 * SQ8)
            q8s = pp.tile([128, HEADS - 2, N], fp8, name="q8s")  # heads 2-7
            k8d = pp.tile([128, HEADS - 2, 2, N], fp8, name="k8d")  # heads 2-7
            # d_sb borrows the pair-3 exp-staging buffers: it is dead by
            # ~13us, long before the first sc staging tile (~55us)
            d_sb = sp.tile([128, 2, 512], bf16, tag="sc", name="d_sb")
            nc.gpsimd.memset(d_sb[:, :, :], 0.0)
            dT_sb = pp.tile([128, NT, 8], f32, name="dT_sb")
            bqp8 = pp.tile([128, 2, 96], fp8, name="bqp8")
            nc.gpsimd.dma_start(out=bqp8[:, :, :], in_=bqp8_d.rearrange("p (t c) -> p t c", t=2))
            # pair-layout quant scratch aliases outT_hi/outT_lo: quant writes
            # + repack reads end ~45us; outT writes begin ~60us, and the tile
            # tracker orders them.  Slot 2p+hl of outT_hi = q pair p hi/lo;
            # outT_lo likewise for k.
            dps = {}
            tps = {}

            def qkproj_half(ps, m, nh, kp, start, stop):
                sb = M2SB[m]
                if sb < 2:
                    whi, wlo, c0 = wqkh_hi, wqkh_lo, sb * 128
                else:
                    whi, wlo, c0 = wqkr_hi, wqkr_lo, (sb - 2) * 128
                terms = [(whi, xc_hi), (wlo, xc_hi), (whi, xc_lo)]
                for i, (wt, xt) in enumerate(terms):
                    nc.tensor.matmul(
                        ps[:, :],
                        lhsT=wt[:, 2 * kp : 2 * kp + 2, c0 : c0 + 128],
                        rhs=xt[:, 2 * kp : 2 * kp + 2, nh * 512 : (nh + 1) * 512],
                        start=(start and i == 0),
                        stop=(stop and i == len(terms) - 1),
                        perf_mode=DR,
                    )

            quant_done = {1: 0, 2: 0, 3: 0}

            def pair_repack(p):
                # d_j = bq.k_j for both heads of pair p from the pair-layout
                # k8 (one DR matmul per nh into a 32p-aligned psum slot), then
                # 12 SBUF->SBUF gpsimd-queue DMAs repack the pair-layout
                # quants into the per-head score-matmul layouts.
                g = 32 * (p - 1)  # psum slot base: only 0/32/64 are legal
                for nh in range(2):
                    if nh not in dps:
                        dps[nh] = ps_o.tile([128, 512], f32, tag="o", name=f"dps{nh}")
                    # hi-term only: the bq.k_lo contribution to the exp
                    # bias is ~3e-4 of score scale -- negligible
                    nc.tensor.matmul(
                        dps[nh][g : g + 32, :],
                        lhsT=bqp8[:, 0, 32 * (p - 1) : 32 * p],
                        rhs=outT_lo[:, 2 * p, nh * 512 : (nh + 1) * 512],
                        start=True, stop=True,
                    )
                    nc.vector.tensor_copy(
                        d_sb[g : g + 2, nh, :], dps[nh][g : g + 2, :]
                    )
                for half in (0, 1):
                    cs = slice(half * 512, (half + 1) * 512)
                    for r in range(2):
                        h = 2 * p + r
                        src_k_hi = outT_lo[r * 64 : (r + 1) * 64, 2 * p, cs]
                        src_k_lo = outT_lo[r * 64 : (r + 1) * 64, 2 * p + 1, cs]
                        nc.gpsimd.dma_start(out=k8d[0:64, h - 2, 0, cs], in_=src_k_hi)
                        nc.gpsimd.dma_start(out=k8d[64:128, h - 2, 0, cs], in_=src_k_hi)
                        nc.gpsimd.dma_start(out=k8d[0:64, h - 2, 1, cs], in_=src_k_lo)
                        nc.gpsimd.dma_start(out=k8d[64:128, h - 2, 1, cs], in_=src_k_lo)
                    if half == 0:
                        for r in range(2):
                            h = 2 * p + r
                            nc.gpsimd.dma_start(
                                out=q8s[0:64, h - 2, :],
                                in_=outT_hi[r * 64 : (r + 1) * 64, 2 * p, :],
                            )
                            nc.gpsimd.dma_start(
                                out=q8s[64:128, h - 2, :],
                                in_=outT_hi[r * 64 : (r + 1) * 64, 2 * p + 1, :],
                            )
                if p == 3:
                    # all pairs' d rows are in d_sb: transpose to per-j bias
                    # columns and scale to exp-argument units
                    for nh in range(2):
                        tps[nh] = ps_o.tile([128, 512], bf16, tag="o", name=f"tps{nh}")
                        for b in range(4):
                            nc.tensor.transpose(
                                tps[nh][:, b * 128 : (b + 1) * 128],
                                d_sb[:, nh, b * 128 : (b + 1) * 128],
                                ident[:, :],
                            )
                        src = tps[nh].rearrange("p (j g s) -> p j g s", j=4, g=4)
                        dst = dT_sb[:, nh * 4 : (nh + 1) * 4, 2:8].rearrange(
                            "p j (g r) -> p j g r", g=3
                        )
                        nc.vector.tensor_scalar(
                            dst, src[:, :, 0:3, 0:2], SCALE / (SBQ * SQ8),
                            None, op0=mult,
                        )

            def qkproj_out(ps, m, nh, eng="dve"):
                if m in (0, 4):
                    # pair-0 bf16 path (biased)
                    dst = qkT[:, m // 4, nh * 512 : (nh + 1) * 512]
                    if eng == "act":
                        nc.scalar.activation(
                            dst, ps[:, :], Ident,
                            bias=bqk_col[:, m : m + 1], scale=UNSC_QK,
                        )
                    else:
                        nc.vector.tensor_scalar(
                            dst, ps[:, :], UNSC_QK, bqk_col[:, m : m + 1],
                            op0=mult, op1=add,
                        )
                    return
                # pairs 1-3: unbiased fp8 hi/lo quant into pair-layout slots
                p = m % 4
                buf = outT_hi if m < 4 else outT_lo
                hi = buf[:, 2 * p, nh * 512 : (nh + 1) * 512]
                lo = buf[:, 2 * p + 1, nh * 512 : (nh + 1) * 512]
                nc.vector.tensor_scalar(hi, ps[:, :], SQ8 * UNSC_QK, None, op0=mult)
                nc.vector.scalar_tensor_tensor(
                    lo, ps[:, :], SQ8 * UNSC_QK, hi, op0=mult, op1=subtract
                )
                quant_done[p] += 1
                if quant_done[p] == 4:
                    pair_repack(p)

            def qkproj_step(m, nh):
                ps = ps_mm.tile([128, 512], f32, tag="mm", name=f"psqk{m}_{nh}")
                qkproj_half(ps, m, nh, 0, True, False)
                qkproj_half(ps, m, nh, 1, False, True)
                qkproj_out(ps, m, nh)

            # ---------- v projection (fp8 DoubleRow, 3-term) ----------
            # The "ones" columns carry 1/SO so the PV denominator column is
            # denom/SO; its reciprocal then scales the attention output by SO,
            # pre-scaling it for the fp8 outT split for free.
            v_aug = pp.tile([128, NT, HEADS * VSTRIDE], bf16)
            nc.gpsimd.memset(
                v_aug.rearrange("p t (h w) -> p t h w", w=VSTRIDE)[:, :, :, 128:130],
                1.0 / SO,
            )
            out_sb = pp.tile([128, NT, D], bf16, name="out_sb")
            exps = {}

            def vproj_step(i):
                # i in [0, 16): t-tile i%8, output half i//8.  Term order
                # matches DMA order: all hi*hi, then wv_lo terms, then x_lo.
                t, nh = i % NT, i // NT
                ps = ps_mm.tile([128, 512], f32, tag="mm", name=f"psv{t}_{nh}")
                terms = []
                for wt, lo_x in ((wv_hi, False), (wv_lo, False), (wv_hi, True)):
                    for kp in range(4):
                        if lo_x:
                            xt = xe_lo if kp < 2 else xc_lo
                        else:
                            xt = xe_hi if kp < 2 else xc_hi
                        terms.append((kp, kp % 2, wt, xt))
                for i2, (kp, kk, wt, xt) in enumerate(terms):
                    nc.tensor.matmul(
                        ps[:, :],
                        lhsT=xt[:, 2 * kk : 2 * kk + 2, t * 128 : (t + 1) * 128],
                        rhs=wt[:, 2 * kp : 2 * kp + 2, nh * 512 : (nh + 1) * 512],
                        start=(i2 == 0),
                        stop=(i2 == len(terms) - 1),
                        perf_mode=DR,
                    )
                dst = v_aug[:, t, nh * 4 * VSTRIDE : (nh + 1) * 4 * VSTRIDE]
                dst = dst.rearrange("p (h w) -> p h w", w=VSTRIDE)[:, :, 0:128]
                nc.vector.tensor_scalar(
                    dst,
                    ps[:, :].rearrange("p (h w) -> p h w", w=128),
                    UNSC_V,
                    None,
                    op0=mult,
                )

            # ---------- scores + exp ----------
            # pair 0: bf16 (K=64 per head); pairs 1-3: fp8 DR at half cost
            def qkt_step(pair, jt, exp2x=False):
                h0, h1 = 2 * pair, 2 * pair + 1
                pss = {
                    h: ps_s.tile([128, N], f32, tag="s", name=f"psS{h}_{jt}")
                    for h in (h0, h1)
                }
                for nh in range(2):
                    for h in (h0, h1):
                        if pair == 0:
                            pr = (h % 2) * 64
                            nc.tensor.matmul(
                                pss[h][:, nh * 512 : (nh + 1) * 512],
                                lhsT=qkT[pr : pr + 64, 1, jt * 128 : (jt + 1) * 128],
                                rhs=qkT[pr : pr + 64, 0, nh * 512 : (nh + 1) * 512],
                                start=True,
                                stop=True,
                            )
                        else:
                            rhs = q8s[:, h - 2, nh * 512 : (nh + 1) * 512].rearrange(
                                "p (o n) -> p o n", o=1
                            ).broadcast_to([128, 2, 512])
                            nc.tensor.matmul(
                                pss[h][:, nh * 512 : (nh + 1) * 512],
                                lhsT=k8d[:, h - 2, :, jt * 128 : (jt + 1) * 128],
                                rhs=rhs,
                                start=True,
                                stop=True,
                                perf_mode=DR,
                            )
                scl = SCALE if pair == 0 else SC8
                for h in (h0, h1):
                    bkw = {} if pair == 0 else {"bias": dT_sb[:, jt, h : h + 1]}
                    if exp2x:
                        # pair 3 is ACT-exp-paced: stage the scores through a
                        # Pool-engine bf16 copy so the exp runs in the 2-byte
                        # 2x ACT mode (612ns vs 1038ns per head-tile) and the
                        # score psum recycles at the copy, not the exp
                        sc = sp.tile([128, N], bf16, tag="sc", name=f"sc{h}_{jt}")
                        nc.gpsimd.tensor_copy(sc[:, :], pss[h][:, :])
                        nc.scalar.activation(
                            exps[h][:, jt, :], sc[:, :], Exp, scale=scl, **bkw
                        )
                    else:
                        nc.scalar.activation(
                            exps[h][:, jt, :], pss[h][:, :], Exp, scale=scl, **bkw
                        )

            def pv_open(pair, s, alt_pool=False, jc_hi=NT):
                # open the PV psum group for (head, i-tile) and emit the
                # matmuls for j-chunks [0, jc_hi) -- the early chunks' exps
                # land jt by jt, so partial emission can fill the wait for
                # the pair's final exps
                h, it = 2 * pair + s // NT, s % NT
                pool, tag = (ps_mm, "mm") if alt_pool and s % 2 else (ps_o, "o")
                pso = pool.tile([128, 129], f32, tag=tag, name=f"psO{h}_{it}")
                for jc in range(jc_hi):
                    nc.tensor.matmul(
                        pso[:, :],
                        lhsT=exps[h][:, jc, it * 128 : (it + 1) * 128],
                        rhs=v_aug[:, jc, h * VSTRIDE : h * VSTRIDE + 129],
                        start=(jc == 0),
                        stop=(jc == NT - 1),
                    )
                return pso

            def pv_finish(pair, s, pso, on_act=False, jc_lo=NT):
                h, it = 2 * pair + s // NT, s % NT
                for jc in range(jc_lo, NT):
                    nc.tensor.matmul(
                        pso[:, :],
                        lhsT=exps[h][:, jc, it * 128 : (it + 1) * 128],
                        rhs=v_aug[:, jc, h * VSTRIDE : h * VSTRIDE + 129],
                        start=False,
                        stop=(jc == NT - 1),
                    )
                recip = sp.tile([128, 1], f32, tag="recip", name=f"rc{h}_{it}", bufs=6)
                dst = out_sb[:, it, h * DV : (h + 1) * DV]
                nc.vector.reciprocal(recip[:, :], pso[:, 128:129])
                if on_act:
                    nc.scalar.activation(dst, pso[:, 0:DV], Ident, scale=recip[:, :])
                else:
                    nc.vector.tensor_scalar(
                        dst, pso[:, 0:DV], recip[:, :], None, op0=mult
                    )

            def pv_step(pair, s, on_act=False, alt_pool=False):
                pso = pv_open(pair, s, alt_pool=alt_pool)
                pv_finish(pair, s, pso, on_act=on_act)

            def outT_step(g, it, pool=None, tag="mm", hi_act=None):
                # transpose heads 4g..4g+3 of i-tile `it` (already SO-scaled),
                # add SO*bv, then split into fp8 hi/lo for the y projection
                pst = (pool or ps_mm).tile(
                    [128, 4, 128], bf16, tag=tag, name=f"psoT{g}_{it}"
                )
                for k in range(4):
                    c = g * 4 + k
                    nc.tensor.transpose(
                        pst[:, k, :],
                        out_sb[:, it, c * 128 : (c + 1) * 128],
                        ident[:, :],
                    )
                # bv is folded into bo on the host (bo' = bv@Wo + bo), so the
                # split is just hi = fp8(psum), lo = fp8(psum - hi), done as
                # single 512-wide ops (narrow DVE ops are overhead-bound).
                # g=0 runs inside the PV(3) stretch where ACT is normalizing
                # PV outputs, so its hi quant goes to DVE; g=1 rides ACT.
                hi_dst = outT_hi[:, g * 4 : (g + 1) * 4, it * 128 : (it + 1) * 128]
                use_act = g == 1 if hi_act is None else hi_act
                hi_eng = nc.scalar.copy if use_act else nc.vector.tensor_copy
                hi_eng(hi_dst, pst[:, :, :])
                nc.vector.tensor_tensor(
                    outT_lo[:, g * 4 : (g + 1) * 4, it * 128 : (it + 1) * 128],
                    pst[:, :, :],
                    hi_dst,
                    op=subtract,
                )

            # ---------- emission schedule ----------
            # prologue: six qkT steps split into 6 term phases pipelined
            # 6-deep (2 ps_mm banks + 2 ps_o banks + 2 ps_s banks, all idle
            # this early) so each matmul only depends on already-landed DMA
            # chunks: phases 0-1 need the hi chunks, 2-3 add wqk_lo, 4-5 xc_lo
            # ps_s holds the two steps whose copy-outs ride ACT (done
            # ~9.3us) so pair-0 scores get their psums early; the m=1/5
            # steps (DVE-quant copy-outs, done ~10.3us) sit in ps_o, which
            # is not needed again until the d-path (~11us)
            PRO = [(0, 0), (4, 0), (0, 1), (4, 1), (1, 0), (5, 0)]
            pro_ps = [
                ps_mm.tile([128, 512], f32, tag="mm", name="psqk0_0"),
                ps_mm.tile([128, 512], f32, tag="mm", name="psqk4_0"),
                ps_s.tile([128, 512], f32, tag="s", name="psqk0_1"),
                ps_s.tile([128, 512], f32, tag="s", name="psqk4_1"),
                ps_o.tile([128, 512], f32, tag="o", name="psqk1_0"),
                ps_o.tile([128, 512], f32, tag="o", name="psqk5_0"),
            ]
            PHASES = [
                (0, True, xc_hi), (1, True, xc_hi),
                (0, False, xc_hi), (1, False, xc_hi),
                (0, True, xc_lo), (1, True, xc_lo),
            ]
            for pi, (kp, use_hi, xt) in enumerate(PHASES):
                for si, (m, nh) in enumerate(PRO):
                    sb = M2SB[m]
                    if sb < 2:
                        wt = wqkh_hi if use_hi else wqkh_lo
                        c0 = sb * 128
                    else:
                        wt = wqkr_hi if use_hi else wqkr_lo
                        c0 = (sb - 2) * 128
                    nc.tensor.matmul(
                        pro_ps[si][:, :],
                        lhsT=wt[:, 2 * kp : 2 * kp + 2, c0 : c0 + 128],
                        rhs=xt[:, 2 * kp : 2 * kp + 2, nh * 512 : (nh + 1) * 512],
                        start=(pi == 0),
                        stop=(pi == len(PHASES) - 1),
                        perf_mode=DR,
                    )
            # copy-outs split across ACT and DVE so the four outs gating
            # pair-0 scores complete in two parallel pairs, not one chain
            for si, (m, nh) in enumerate(PRO):
                # pair-0 outs split 2+2 across ACT and DVE (both idle
                # here) so all four finish ~8.2us and ps_s frees for the
                # first scores; the m=1/5 quant ops queue after on DVE
                qkproj_out(pro_ps[si], m, nh, eng="act" if si < 2 else "dve")

            QKT_REST = [(1, 1), (5, 1), (2, 0), (6, 0),
                        (2, 1), (6, 1), (3, 0), (7, 0), (3, 1), (7, 1)]
            for pair in range(HEADS // 2):
                h0, h1 = 2 * pair, 2 * pair + 1
                exps[h0] = ep.tile([128, NT, N], bf16, tag="expS", name=f"eS{h0}")
                exps[h1] = ep.tile([128, NT, N], bf16, tag="expS", name=f"eS{h1}")
                for jt in range(NT):
                    qkt_step(pair, jt)
                    if pair == 0:
                        # jt 0-4: remaining qkT-projection steps (qk data has
                        # landed; v data is still streaming in)
                        if jt < 5:
                            qkproj_step(*QKT_REST[2 * jt])
                            qkproj_step(*QKT_REST[2 * jt + 1])
                        elif jt >= 5:
                            # jt5-6 slots are empty and run ~20-23us, after
                            # wv_lo (~16.4) and xe_lo (~17.9) land: one vproj
                            # each fills the PE rotation gaps there.  jt7
                            # keeps exactly two steps for the pair-0 ->
                            # pair-1 boundary bubble (~2.6us of ps_s idle)
                            if jt == 7:
                                vproj_step(0)
                                vproj_step(1)
                            else:
                                vproj_step(jt - 3)
                    elif pair == 1:
                        if jt < 4:
                            vproj_step(4 + jt)
                        elif jt >= 4:
                            for q in range(4):
                                pv_step(0, 4 * (jt - 4) + q, alt_pool=True)
                    elif pair == 2:
                        if jt < 3:
                            vproj_step(8 + 2 * jt)
                            vproj_step(9 + 2 * jt)
                        elif jt >= 4:
                            for q in range(4):
                                pv_step(1, 4 * (jt - 4) + q, alt_pool=True)
                    elif jt < 2:
                        # vproj 14-15 ride pair-3's light slots; PV(2) waits
                        # until jt2 so its matmuls never stall on the v15
                        # copy-out in PE program order
                        vproj_step(14 + jt)
                    elif jt < 4:
                        pv_step(2, 3 * (jt - 2))
                        pv_step(2, 3 * (jt - 2) + 1)
                        pv_step(2, 3 * (jt - 2) + 2)
                    elif jt < 6:
                        pv_step(2, 6 + 3 * (jt - 4))
                        pv_step(2, 7 + 3 * (jt - 4))
                        pv_step(2, 8 + 3 * (jt - 4))
                        outT_step(0, jt - 4)
                    elif jt == 6:
                        pv_step(2, 12)
                        pv_step(2, 13)
                        outT_step(0, 2)
                    else:
                        pv_step(2, 14)
                        pv_step(2, 15)
                        outT_step(0, 3)


            # ---------- y = outT^T @ Wo + bo (fp8 DoubleRow, 3-term) ----------
            def yproj(mt, nh, c0, c1, fast_tail=False, dma_eng=None):
                # y columns [c0*128, c1*128) of token tile mt
                w = (c1 - c0) * 128
                y_tile = yp.tile([128, 512], f32, tag="y", name=f"y{mt}_{nh}_{c0}")
                if not fast_tail and nh == 0:
                    yt = sp.tile([128, 512], f32, tag="sc", name=f"yt{mt}_{nh}_{c0}")
                ps = ps_s.tile([128, 512], f32, tag="s", name=f"psy{mt}_{nh}_{c0}")
                terms = [(outT_hi, wo_hi), (outT_lo, wo_hi), (outT_hi, wo_lo)]
                for i2, (ot, wt) in enumerate(terms):
                    for kp in range(4):
                        nc.tensor.matmul(
                            ps[:, 0:w],
                            lhsT=ot[:, 2 * kp : 2 * kp + 2, mt * 128 : (mt + 1) * 128],
                            rhs=wt[:, 2 * kp : 2 * kp + 2, c0 * 128 : c1 * 128],
                            start=(i2 == 0 and kp == 0),
                            stop=(i2 == 2 and kp == 3),
                            perf_mode=DR,
                        )
                if fast_tail or nh == 1:
                    # single fused DVE op (Pool cannot read PSUM)
                    nc.vector.scalar_tensor_tensor(
                        y_tile[:, 0:w], ps[:, 0:w], UNSC_Y,
                        bo_bc[:, c0 * 128 : c1 * 128], op0=mult, op1=add,
                    )
                else:
                    # ACT unscale first: frees the scores-pool psum slot
                    # quickly so the next yproj group isn't held up
                    nc.scalar.activation(yt[:, 0:w], ps[:, 0:w], Ident, scale=UNSC_Y)
                    nc.vector.tensor_tensor(
                        y_tile[:, 0:w], yt[:, 0:w], bo_bc[:, c0 * 128 : c1 * 128],
                        op=add,
                    )
                # the very last pieces trigger their DMA from the DVE
                # queue (the stt producer): no cross-engine hop, and no
                # queueing behind the SP sequencer's pending sem-waits
                dq = dma_eng or nc.sync
                dq.dma_start(
                    out=y_d[mt * 128 : (mt + 1) * 128, c0 * 128 : c1 * 128],
                    in_=y_tile[:, 0:w],
                )

            # ---------- fused tail: PV(3) it-major + transposes + yproj ----
            # outT(0, 0..3) (heads 0-3, ready since pair 2) fill the wait for
            # the last pair-3 exps; then each iteration finishes both heads of
            # PV(3) for one i-tile, transposes it, and runs the y projection
            # one tile behind -- so the write stream starts ~7us earlier and
            # PV(3)'s copy-out latency hides under yproj matmuls.
            for it in range(4, 6):
                outT_step(0, it)
            # the first four PV(3) groups pre-emit their first 7 j-chunk
            # matmuls (those exps landed jt-by-jt during pair 3); only the
            # jc=7 matmuls wait on the pair's final exps, so the PE stays fed
            # across the exp-chain handoff
            pre = {s: pv_open(3, s, alt_pool=True, jc_hi=NT - 1)
                   for s in (0, 8, 1, 9)}
            for it in range(NT):
                if it < 2:
                    pv_finish(3, it, pre[it], jc_lo=NT - 1, on_act=True)
                    # heads 0-3 transposes for the last i-tiles fill the
                    # waits on the pair's final exps
                    outT_step(0, 6 + it)
                    pv_finish(3, 8 + it, pre[8 + it], jc_lo=NT - 1, on_act=True)
                else:
                    pv_step(3, it, alt_pool=True)
                    pv_step(3, 8 + it, alt_pool=True)
                outT_step(1, it)
                if it >= 1:
                    # the last two in-loop tiles also take the short-chain
                    # path so their writes issue promptly ahead of the finale
                    ft = it >= NT - 2
                    yproj(it - 1, 0, 0, 4, fast_tail=ft)
                    yproj(it - 1, 1, 4, 8, fast_tail=ft)
            yproj(NT - 1, 0, 0, 4, fast_tail=True)
            # keep the closing matmul->bias->DMA chain short, but pay the
            # per-DMA overhead (625ns HWDGE + SP seq) only twice more:
            # one 384-wide chunk, then a 128-wide finale
            yproj(NT - 1, 1, 4, 7, fast_tail=True)
            yproj(NT - 1, 1, 7, 8, fast_tail=True)

    nc.finalize()
    return nc


def _get_compiled():
    global _COMPILED
    if _COMPILED is None:
        _COMPILED = _build()
    return _COMPILED


def _split8(a, s):
    scaled = np.asarray(a, np.float32) * s
    hi = scaled.astype(E4M3)
    lo = (scaled - hi.astype(np.float32)).astype(E4M3)
    return hi, lo


def _bqp8(bqk):
    bq = bqk[0:512].reshape(8, 64)
    out = np.zeros((128, 192), np.float32)
    for p in range(1, 4):
        for r in range(2):
            for t in range(2):
                col = t * 96 + 32 * (p - 1) + r
                out[r * 64 : (r + 1) * 64, col] = bq[2 * p + r] * SBQ_H
    return out.astype(E4M3)


def _sbuf_pack(a):
    """[C*128, N] -> [128, C*N]: the on-chip layout, so DMAs are straight
    maximal-contiguous copies."""
    c = a.shape[0] // 128
    return np.ascontiguousarray(
        a.reshape(c, 128, a.shape[1]).transpose(1, 0, 2).reshape(128, -1)
    )


def _prep_inputs(inputs: dict) -> list:
    """Per-core DRAM-parameter dicts (host-side prep: transpose + fp8 split)."""
    x = np.ascontiguousarray(np.asarray(inputs["x"], np.float32))
    wqk = np.asarray(inputs["Wqk"], np.float32)
    # column m-blocks permuted so the blocks gating attention pair 0 (m=0,4)
    # sit first and ride the first, smallest DMA
    perm = np.concatenate([np.arange(m * 128, (m + 1) * 128) for m in M_ORDER])
    wqk_hi, wqk_lo = _split8(wqk[:, perm], SQK)
    wv_hi, wv_lo = _split8(inputs["Wv"], SV)
    wo_hi, wo_lo = _split8(inputs["Wo"], SY)
    shared = {
        "wqkh_hi": _sbuf_pack(wqk_hi[:, 0:256]),
        "wqkh_lo": _sbuf_pack(wqk_lo[:, 0:256]),
        "wqkr_hi": _sbuf_pack(wqk_hi[:, 256:1024]),
        "wqkr_lo": _sbuf_pack(wqk_lo[:, 256:1024]),
        "wv_hi": _sbuf_pack(wv_hi),
        "wv_lo": _sbuf_pack(wv_lo),
        "wo_hi": _sbuf_pack(wo_hi),
        "wo_lo": _sbuf_pack(wo_lo),
        # [m-block, partition] -> [partition, m-block], host-packed so the
        # device DMA is a straight 32B-per-line copy, not a 4B-element gather
        "bqk_t": np.ascontiguousarray(
            np.asarray(inputs["bqk"], np.float32).reshape(8, 128).T
        ),
        # bq (q bias) per head of pairs 1-3, fp8-scaled, in the pair-layout
        # partition rows, duplicated across the two DR k-tiles: feeds the
        # on-device d_j = bq.k_j exp-bias matmul
        "bqp8": _bqp8(np.asarray(inputs["bqk"], np.float64)),
        # softmax rows sum to 1, so bv passes through attention unchanged and
        # can be folded into bo: y = out@Wo + (bv@Wo + bo)
        "bo_t": np.broadcast_to(
            (np.asarray(inputs["bv"], np.float64) @ np.asarray(inputs["Wo"], np.float64)
             + np.asarray(inputs["bo"], np.float64)).astype(np.float32).astype(BF16),
            (128, D),
        ).copy(),
    }
    in_maps = []
    for b in range(B):
        xT = np.ascontiguousarray(x[b].T)          # [feat, tok]
        xh, xl = _split8(xT, SX)
        in_maps.append({
            "xTe_hi": _sbuf_pack(xh[:CLS]),
            "xTe_lo": _sbuf_pack(xl[:CLS]),
            "xTc_hi": _sbuf_pack(xh[CLS:]),
            "xTc_lo": _sbuf_pack(xl[CLS:]),
            **shared,
        })
    return in_maps


def _run(inputs: dict, trace: bool = False):
    from concourse.bass_utils import run_bass_kernel_spmd

    nc = _get_compiled()
    in_maps = _prep_inputs(inputs)
    res = run_bass_kernel_spmd(nc, in_maps, core_ids=list(range(B)), trace=trace)
    y = np.stack([res.results[b]["y"] for b in range(B)], axis=0)
    return y, res


def kernel(**inputs) -> np.ndarray:
    # The axon/NRT stack occasionally throws transient errors (compile-hook
    # INTERNAL hiccups, NRT_EXEC_UNIT_UNRECOVERABLE on a wedged device);
    # both have always succeeded on a plain retry.
    import time as _time

    last = None
    for attempt in range(3):
        try:
            y, _ = _run(inputs, trace=False)
            return y
        except Exception as e:  # noqa: BLE001 - re-raised after retries
            last = e
            if attempt < 2:
                _time.sleep(3.0)
    raise last

